# revision 1
# baseline (speedup 1.0000x reference)
# AMCNN Trainium2 kernel: 8-core data-parallel (batch sharded 16/core).
# GRU layers computed by global fixed-point iteration (K iterations of
# big matmuls + elementwise + tensor_tensor_scan for the h-recurrence).
# Self-contained: builds and runs the Bass program via run_bass_kernel_spmd.
import numpy as np
import ml_dtypes


def _ensure_paths():
    import sys
    try:
        import concourse  # noqa: F401
        return
    except ImportError:
        pass
    for p in ('/root/.axon_site', '/root/.axon_site/_ro/trn_rl_repo',
              '/root/.axon_site/_ro/pypackages', '/opt/trn_rl_repo', '/opt/pypackages'):
        if p not in sys.path:
            sys.path.append(p)
    import concourse  # noqa: F401


_ensure_paths()

import concourse.bass as bass            # noqa: E402
import concourse.mybir as mybir          # noqa: E402
import concourse.tile as tile            # noqa: E402
from concourse.bass_utils import run_bass_kernel_spmd  # noqa: E402
from concourse.vector_clock import ScopedClock         # noqa: E402

BF = mybir.dt.bfloat16
F32 = mybir.dt.float32
I32 = mybir.dt.int32
AF = mybir.ActivationFunctionType
OP = mybir.AluOpType
bf16 = ml_dtypes.bfloat16

# model dims
B, L, V, E, H, C, OUT = 128, 256, 50000, 300, 128, 4, 2
D = 2 * H
EP = 384            # E padded to 3*128
NCORES = 8
BS = B // NCORES    # 16 utterances per core
NTOK = BS * L       # 4096 tokens per core
KIT = 5             # fixed-point iterations per GRU layer
FS = (2, 3, 4, 5)
BLKOFF = [0, 4, 13, 29]    # offsets of (o,di) blocks within the 54-wide combo axis
MOFF = [0, 2, 5, 9]        # offsets of o within the 14-wide output axis
NQ = 260                   # Q free width per batch (256 tokens + 4 guard cols)

DEBUG = False
TRACE = False
_last_debug = None
_last_exec_ns = None


# ---------------------------------------------------------------- patches
def _install_patches():
    """Two workarounds for this walrus build, which rejects more than one
    semaphore wait on a single instruction ("Too many sync wait commands")."""
    def _drain_and_barrier(self, tick_clock, wait_clock):
        probe = self.nc.sync.nop(hint="tile_exit_wait_probe", nofuse=True)
        wait_clock.add_sem_waits(probe.ins, ScopedClock({None: tick_clock.global_clock}))
        si = probe.ins.sync_info
        waits = list(si.on_wait or []) if si else []
        if len(waits) > 1:
            si.on_wait = waits[:1]
            for w in waits[1:]:
                n2 = self.nc.sync.nop(hint="tile_exit_wait_split", nofuse=True)
                if n2.ins.sync_info is None:
                    n2.ins.sync_info = mybir.SyncInfo(on_wait=[], on_update=[])
                n2.ins.sync_info.on_wait = [w]
        self.nc.sync.drain()
        self.nc.all_engine_barrier()
        assert self.sems is not None
        popped = self.nc._tile_sem_poison_stack.pop()
        assert popped is self._sem_poison
        self.nc.clear_and_free_semaphores(list(self.sems.allocated().values()))
        self.nc.all_engine_barrier()
    tile.TileContext._drain_and_barrier = _drain_and_barrier


_install_patches()
_ws_ctr = [0]


def _split_waits(nc):
    for f in nc.m.functions:
        for bb in f.blocks:
            insts = bb.instructions
            i = 0
            while i < len(insts):
                ins = insts[i]
                si = ins.sync_info
                waits = list(si.on_wait) if (si and si.on_wait) else []
                if len(waits) > 1:
                    si.on_wait = waits[-1:]
                    for w in waits[:-1]:
                        _ws_ctr[0] += 1
                        nop = mybir.InstNoOp(
                            name=f"waitsplit-{_ws_ctr[0]}", engine=ins.engine,
                            ins=[], outs=[],
                            sync_info=mybir.SyncInfo(on_wait=[w], on_update=[]))
                        insts.insert(i, nop)
                        i += 1
                i += 1
    return nc


# ---------------------------------------------------------------- builder
def _build_nc():
    nc = bass.Bass()
    P = {}

    def inp(name, shape, dt):
        P[name] = nc.declare_dram_parameter(name, list(shape), dt, isOutput=False)
        return P[name]

    def outp(name, shape, dt):
        P[name] = nc.declare_dram_parameter(name, list(shape), dt, isOutput=True)
        return P[name]

    inp('emb16', (V, EP), BF)
    inp('idx', (NTOK, 1), I32)
    inp('w0t', (128, 2, 3, EP), BF)
    inp('w1t', (128, 2, 2, EP), BF)
    inp('whh', (128, 2, 2, 3, 128), BF)
    inp('gb', (128, 2, 2, 4), F32)
    inp('awT', (128, 4, 2, 256), BF)
    inp('av2T', (128, 4, 2, 256), BF)
    inp('abv', (128, 4, 2), F32)
    inp('abc', (128, 4), F32)
    inp('wv1', (128, 4, 2), BF)
    inp('wc', (128, 2, 2, 108), BF)
    inp('sel', (128, 5, 2, 14), BF)
    inp('cb14', (16, 14), F32)
    inp('fcw', (16, 2), BF)
    inp('fcb', (2, 1), F32)
    inp('identb', (128, 128), BF)
    inp('identf', (128, 128), F32)
    inp('ones1', (128, 1), F32)
    inp('onesr', (1, 128), F32)
    inp('bind', (2, 108), F32)
    outp('out', (2, 16), F32)
    if DEBUG:
        for nm in ('d_hs0f', 'd_hs0b', 'd_hs1f', 'd_hs1b'):
            outp(nm, (128, 16, 256), BF)
        outp('d_Aacc', (128, 2, 64), F32)
        outp('d_aik', (128, 2, 64), F32)
        outp('d_aa', (64, 256), BF)
        outp('d_maxy', (14, 16), F32)
        outp('d_gx0', (128, 2, 3, 16, 256), BF)
        outp('d_zs0', (128, 16, 256), BF)
        outp('d_nbs0', (128, 16, 256), BF)
        outp('d_x2f', (128, 16, 256), BF)
        outp('d_x2b', (128, 16, 256), BF)

    tc_cm = tile.TileContext(nc)
    tc = tc_cm.__enter__()
    try:
        _emit(nc, tc, P)
    finally:
        tc_cm.__exit__(None, None, None)
    _split_waits(nc)
    return nc


def _emit(nc, tc, P):
    import contextlib
    ctx = contextlib.ExitStack()
    with ctx:
        pers = ctx.enter_context(tc.tile_pool(name="pers", bufs=1))
        matg = ctx.enter_context(tc.tile_pool(name="matg", bufs=2))
        tp = ctx.enter_context(tc.tile_pool(name="tp", bufs=2))
        tp3 = ctx.enter_context(tc.tile_pool(name="tp3", bufs=3))
        psum = ctx.enter_context(tc.tile_pool(name="psum", bufs=3, space="PSUM"))
        psumb = ctx.enter_context(tc.tile_pool(name="psumb", bufs=2, space="PSUM"))

        # ---- load weights / consts to SBUF
        def load(name, shape, dt, tag=None):
            t = pers.tile(list(shape), dt, tag=tag or name)
            nc.sync.dma_start(t[:], P[name][:])
            return t

        w0s = load('w0t', (128, 2, 3, EP), BF)
        w1s = load('w1t', (128, 2, 2, EP), BF)
        whhs = load('whh', (128, 2, 2, 3, 128), BF)
        gbs = load('gb', (128, 2, 2, 4), F32)
        idents = load('identb', (128, 128), BF)
        identf = load('identf', (128, 128), F32)
        ones1 = load('ones1', (128, 1), F32)
        onesr = load('onesr', (1, 128), F32)
        binds = load('bind', (2, 108), F32)

        # ---- persistent activation tensors
        xT = pers.tile([128, 3, BS, L], BF, tag="xT")            # E-major x
        hs = [[pers.tile([128, BS, L], BF, tag=f"hs{l}{d}", name=f"hst{l}{d}")
               for d in (0, 1)] for l in (0, 1)]
        zs = [pers.tile([128, BS, L], BF, tag=f"zs{d}", name=f"zst{d}") for d in (0, 1)]
        nbs = [pers.tile([128, BS, L], BF, tag=f"nbs{d}", name=f"nbst{d}") for d in (0, 1)]
        zero4 = pers.tile([128, 4], BF, tag="zero4")
        nc.gpsimd.memset(zero4[:], 0.0)

        # ---- phase A: gather + transpose + gx0
        it = pers.tile([128, 32], I32, tag="idx")
        nc.sync.dma_start(it[:], P['idx'][:, 0].rearrange("(j p) -> p j", p=128))
        for j in range(32):
            xtm = tp3.tile([128, EP], BF, tag="gath")
            nc.gpsimd.indirect_dma_start(
                out=xtm[:], out_offset=None, in_=P['emb16'][:],
                in_offset=bass.IndirectOffsetOnAxis(ap=it[:, j:j + 1], axis=0))
            b_, lo = j // 2, (j % 2) * 128
            for c in range(3):
                pt = psumb.tile([128, 128], BF, space="PSUM", tag="psb")
                nc.tensor.transpose(pt[:], xtm[:, c * 128:(c + 1) * 128], idents[:])
                nc.scalar.activation(xT[:, c, b_, lo:lo + 128], pt[:], AF.Copy)

        gx = pers.tile([128, 2, 3, BS, L], BF, tag="gx", name="gx0t")
        for d in range(2):
            for g in range(3):
                for bp in range(8):
                    ps = psum.tile([128, 2, L], F32, space="PSUM", tag="ps2")
                    for k in range(3):
                        rhs = xT[:, k, 2 * bp:2 * bp + 2, :]
                        if d == 1:
                            rhs = rhs[:, :, ::-1]
                        nc.tensor.matmul(ps[:], w0s[:, d, k, g * 128:(g + 1) * 128],
                                         rhs, start=(k == 0), stop=(k == 2))
                    nc.scalar.activation(gx[:, d, g, 2 * bp:2 * bp + 2, :], ps[:], AF.Copy)

        # ---- phase B: GRU layers by global fixed point
        for l in range(2):
            if l == 1:
                # rebuild gx from layer-0 output (bwd dir stored position-reversed)
                gx = pers.tile([128, 2, 3, BS, L], BF, tag="gx", name="gx1t")
                for d in range(2):
                    for g in range(3):
                        for bp in range(8):
                            ps = psum.tile([128, 2, L], F32, space="PSUM", tag="ps2")
                            for kd in range(2):
                                if kd == d:
                                    rhs = hs[0][kd][:, 2 * bp:2 * bp + 2, :]
                                else:
                                    rhs = hs[0][kd][:, 2 * bp:2 * bp + 2, ::-1]
                                nc.tensor.matmul(
                                    ps[:], w1s[:, d, kd, g * 128:(g + 1) * 128],
                                    rhs, start=(kd == 0), stop=(kd == 1))
                            nc.scalar.activation(gx[:, d, g, 2 * bp:2 * bp + 2, :],
                                                 ps[:], AF.Copy)
            for itn in range(KIT):
                for d in range(2):
                    for bp in range(4):
                        bs4 = slice(4 * bp, 4 * bp + 4)
                        gxv = [gx[:, d, g_, bs4, :] for g_ in range(3)]
                        if itn == 0:
                            # h == 0: gates come straight from gx (+biases)
                            r_sb = tp.tile([128, 4, L], BF, tag="r")
                            nc.scalar.activation(r_sb[:], gxv[0], AF.Sigmoid,
                                                 bias=gbs[:, l, d, 0:1])
                            nc.scalar.activation(zs[d][:, bs4, :], gxv[1], AF.Sigmoid,
                                                 bias=gbs[:, l, d, 1:2])
                            t1 = tp.tile([128, 4, L], BF, tag="t1")
                            nc.vector.tensor_scalar_mul(t1[:], r_sb[:],
                                                        gbs[:, l, d, 2:3])
                            nc.vector.tensor_tensor(t1[:], t1[:], gxv[2], OP.add)
                        else:
                            hprev = hs[l][d][:, bs4, 0:L - 1]
                            # r gate: psum = gx (I-matmul) + Whh h
                            pr = psum.tile([128, 4, L], F32, space="PSUM", tag="ps2")
                            for j in (0, 1):
                                j2 = slice(2 * j, 2 * j + 2)
                                nc.tensor.matmul(pr[:, j2, :], idents[:],
                                                 gxv[0][:, j2, :],
                                                 start=True, stop=False)
                                nc.tensor.matmul(pr[:, j2, 1:L], whhs[:, l, d, 0, :],
                                                 hprev[:, j2, :],
                                                 start=False, stop=True)
                            r_sb = tp.tile([128, 4, L], BF, tag="r")
                            nc.scalar.activation(r_sb[:], pr[:], AF.Sigmoid,
                                                 bias=gbs[:, l, d, 0:1])
                            pz = psum.tile([128, 4, L], F32, space="PSUM", tag="ps2")
                            for j in (0, 1):
                                j2 = slice(2 * j, 2 * j + 2)
                                nc.tensor.matmul(pz[:, j2, :], idents[:],
                                                 gxv[1][:, j2, :],
                                                 start=True, stop=False)
                                nc.tensor.matmul(pz[:, j2, 1:L], whhs[:, l, d, 1, :],
                                                 hprev[:, j2, :],
                                                 start=False, stop=True)
                            nc.scalar.activation(zs[d][:, bs4, :], pz[:], AF.Sigmoid,
                                                 bias=gbs[:, l, d, 1:2])
                            pn = psum.tile([128, 4, L], F32, space="PSUM", tag="ps2")
                            nc.tensor.matmul(pn[:, :, 0:1], whhs[:, l, d, 2, :],
                                             zero4[:], start=True, stop=True)
                            for j in (0, 1):
                                j2 = slice(2 * j, 2 * j + 2)
                                nc.tensor.matmul(pn[:, j2, 1:L], whhs[:, l, d, 2, :],
                                                 hprev[:, j2, :],
                                                 start=True, stop=True)
                            t1 = tp.tile([128, 4, L], BF, tag="t1")
                            nc.vector.scalar_tensor_tensor(
                                t1[:], pn[:], gbs[:, l, d, 2:3], r_sb[:],
                                OP.add, OP.mult)
                            nc.vector.tensor_tensor(t1[:], t1[:], gxv[2], OP.add)
                        n_sb = tp.tile([128, 4, L], BF, tag="n")
                        nc.scalar.activation(n_sb[:], t1[:], AF.Tanh,
                                             bias=gbs[:, l, d, 3:4])
                        nc.vector.scalar_tensor_tensor(
                            nbs[d][:, bs4, :], zs[d][:, bs4, :], 1.0, n_sb[:],
                            OP.subtract, OP.mult)
                    # chain resets: h_{-1}=0 per batch via z[.,b,0] := 0
                    nc.vector.tensor_scalar_mul(zs[d][:, :, 0:1], zs[d][:, :, 0:1], 0.0)
                    # h_t = z*h - (z-1)*n
                    nc.vector.tensor_tensor_scan(
                        hs[l][d].rearrange("p a b -> p (a b)"),
                        zs[d].rearrange("p a b -> p (a b)"),
                        nbs[d].rearrange("p a b -> p (a b)"),
                        0.0, OP.mult, OP.subtract)
            if DEBUG and l == 0:
                nc.sync.dma_start(P['d_zs0'][:], zs[0][:])
                nc.sync.dma_start(P['d_nbs0'][:], nbs[0][:])

        if DEBUG:
            nc.sync.dma_start(P['d_hs0f'][:], hs[0][0][:])
            nc.sync.dma_start(P['d_hs0b'][:], hs[0][1][:])
            nc.sync.dma_start(P['d_hs1f'][:], hs[1][0][:])
            nc.sync.dma_start(P['d_hs1b'][:], hs[1][1][:])
            nc.sync.dma_start(P['d_gx0'][:], gx[:])

        # ---- x2: real-time-ordered layer-1 output (fwd straight, bwd reversed)
        x2b = pers.tile([128, BS, L], BF, tag="zs1", name="x2b")
        nc.vector.tensor_copy(x2b[:], hs[1][1][:, :, ::-1])
        x2 = [hs[1][0], x2b]
        if DEBUG:
            nc.sync.dma_start(P['d_x2f'][:], x2[0][:])
            nc.sync.dma_start(P['d_x2b'][:], x2[1][:])

        _emit_attention(nc, tc, P, ctx, pers, matg, tp, tp3, psum, psumb, x2,
                        idents, identf, ones1, onesr, binds)


def _emit_attention(nc, tc, P, ctx, pers, matg, tp, tp3, psum, psumb, x2,
                    idents, identf, ones1, onesr, binds):
    aws = pers.tile([128, 4, 2, 256], BF, tag="w0t")
    nc.sync.dma_start(aws[:], P['awT'][:])
    av2s = pers.tile([128, 4, 2, 256], BF, tag="w1t")
    nc.sync.dma_start(av2s[:], P['av2T'][:])
    abvs = pers.tile([128, 4, 2], F32, tag="abv")
    nc.sync.dma_start(abvs[:], P['abv'][:])
    abcs = pers.tile([128, 4], F32, tag="abc")
    nc.sync.dma_start(abcs[:], P['abc'][:])
    wv1s = pers.tile([128, 4, 2], BF, tag="wv1")
    nc.sync.dma_start(wv1s[:], P['wv1'][:])
    wcs = pers.tile([128, 2, 2, 108], BF, tag="wc")
    nc.sync.dma_start(wcs[:], P['wc'][:])
    sels = pers.tile([128, 5, 2, 14], BF, tag="sel")
    nc.sync.dma_start(sels[:], P['sel'][:])
    cb14s = pers.tile([16, 14], F32, tag="cb14")
    nc.sync.dma_start(cb14s[:], P['cb14'][:])
    fcws = pers.tile([16, 2], BF, tag="fcw")
    nc.sync.dma_start(fcws[:], P['fcw'][:])
    fcbs = pers.tile([2, 1], F32, tag="fcb")
    nc.sync.dma_start(fcbs[:], P['fcb'][:])

    Aacc = pers.tile([128, 2, 64], F32, tag="Aacc")
    aaP = pers.tile([64, L], BF, tag="aaP")
    stageF = pers.tile([1, 64 * L], BF, tag="xT")  # reuse dead xT slot

    # ---- per (c,b): mat, M, tanh-accum A ; then g, aa_pre
    for c in range(4):
        for b in range(BS):
            mat_cb = matg.tile([128, 2, L], BF, tag="matcb")
            pm = psum.tile([128, 2, L], F32, space="PSUM", tag="ps2")
            for ech in range(2):
                for kd in range(2):
                    nc.tensor.matmul(pm[:, ech, :],
                                     aws[:, c, kd, ech * 128:(ech + 1) * 128],
                                     x2[kd][:, b, :], start=(kd == 0), stop=(kd == 1))
            nc.vector.tensor_copy(mat_cb[:], pm[:])
            for ich in range(2):
                pM = psum.tile([128, L], F32, space="PSUM", tag="ps2")
                for kch in range(2):
                    nc.tensor.matmul(pM[:], x2[kch][:, b, ich * 128:(ich + 1) * 128],
                                     mat_cb[:, kch, :], start=(kch == 0), stop=(kch == 1))
                tsc = tp.tile([128, L], BF, tag="r")
                nc.scalar.activation(tsc[:], pM[:], AF.Tanh, bias=abcs[:, c:c + 1],
                                     accum_out=Aacc[:, ich, c * 16 + b:c * 16 + b + 1])
    for c in range(4):
        for b in range(BS):
            g_cb = matg.tile([128, 2, L], BF, tag="matcb")
            for ech in range(2):
                pg = psum.tile([128, L], F32, space="PSUM", tag="ps2")
                for kd in range(2):
                    nc.tensor.matmul(pg[:], av2s[:, c, kd, ech * 128:(ech + 1) * 128],
                                     x2[kd][:, b, :], start=(kd == 0), stop=(kd == 1))
                nc.scalar.activation(g_cb[:, ech, :], pg[:], AF.Sigmoid,
                                     bias=abvs[:, c, ech:ech + 1])
            pa = psum.tile([1, L], F32, space="PSUM", tag="ps2")
            for ech in range(2):
                nc.tensor.matmul(pa[:], wv1s[:, c, ech:ech + 1], g_cb[:, ech, :],
                                 start=(ech == 0), stop=(ech == 1))
            cb_ = c * 16 + b
            nc.vector.tensor_copy(stageF[0:1, cb_ * L:(cb_ + 1) * L], pa[:])

    nc.sync.dma_start(aaP[:], stageF[0:1, :].rearrange("p (r l) -> p r l", l=L))

    # ---- softmaxes (exp table set)
    expA = pers.tile([128, 2, 64], F32, tag="expA")
    nc.scalar.activation(expA[:], Aacc[:], AF.Exp)
    psA = psum.tile([1, 64], F32, space="PSUM", tag="ps2")
    for ich in range(2):
        nc.tensor.matmul(psA[:], ones1[:], expA[:, ich, :],
                         start=(ich == 0), stop=(ich == 1))
    sA = tp.tile([1, 64], F32, tag="sA")
    nc.vector.tensor_copy(sA[:], psA[:])
    rA = tp.tile([1, 64], F32, tag="rA")
    nc.vector.reciprocal(rA[:], sA[:])
    aik = pers.tile([128, 2, 64], F32, tag="aik")
    pB = psum.tile([128, 64], F32, space="PSUM", tag="ps2", name="pB")
    nc.tensor.matmul(pB[:], onesr[:], rA[:], start=True, stop=True)
    for ich in range(2):
        nc.vector.tensor_tensor(aik[:, ich, :], expA[:, ich, :], pB[:], OP.mult)

    expaa = pers.tile([64, L], F32, tag="expaa")
    nc.scalar.activation(expaa[:], aaP[:], AF.Exp)
    saa = tp.tile([64, 1], F32, tag="saa")
    nc.vector.reduce_sum(saa[:], expaa[:], axis=mybir.AxisListType.X)
    raa = tp.tile([64, 1], F32, tag="raa")
    nc.vector.reciprocal(raa[:], saa[:])
    aab = pers.tile([64, L], BF, tag="aab")
    nc.vector.tensor_scalar_mul(aab[:], expaa[:], raa[:, 0:1])

    if DEBUG:
        nc.sync.dma_start(P['d_Aacc'][:], Aacc[:])
        nc.sync.dma_start(P['d_aik'][:], aik[:])
        nc.sync.dma_start(P['d_aa'][:], aab[:])

    # ---- a transposed to (cb, i) then replicated to the conv-combo axis
    aT_cb = pers.tile([64, 2, 128], F32, tag="aTcb")
    for ich in range(2):
        pT = psum.tile([64, 128], F32, space="PSUM", tag="ps2")
        nc.tensor.transpose(pT[:], aik[:, ich, :], identf[:])
        nc.vector.tensor_copy(aT_cb[:, ich, :], pT[:])

    # ---- aaT: (l, lch, cb) for the 'new' matmuls
    aaT = pers.tile([128, 2, 64], BF, tag="aaT")
    for lch in range(2):
        pT = psumb.tile([128, 64], BF, space="PSUM", tag="psb")
        nc.tensor.transpose(pT[:], aab[:, lch * 128:(lch + 1) * 128],
                            idents[0:64, 0:64])
        nc.vector.tensor_copy(aaT[:, lch, :], pT[:])

    # ---- xTM: token-major x2 (l on partitions)
    xTM = pers.tile([128, 2, BS, 256], BF, tag="gx")
    for dch in range(2):
        for b in range(BS):
            for lch in range(2):
                pT = psumb.tile([128, 128], BF, space="PSUM", tag="psb")
                nc.tensor.transpose(pT[:], x2[dch][:, b, lch * 128:(lch + 1) * 128],
                                    idents[:])
                nc.vector.tensor_copy(xTM[:, lch, b, dch * 128:(dch + 1) * 128],
                                       pT[:])

    # ---- new[c,b,d] then P2 const
    news = pers.tile([128, 2, BS, 4], BF, tag="news")
    for b in range(BS):
        for dch in range(2):
            pn = psum.tile([128, 4], F32, space="PSUM", tag="ps2")
            for lch in range(2):
                nc.tensor.matmul(pn[:], xTM[:, lch, b, dch * 128:(dch + 1) * 128],
                                 aaT[:, lch, b::16], start=(lch == 0), stop=(lch == 1))
            nc.vector.tensor_copy(news[:, dch, b, :], pn[:])

    pP2 = psum.tile([16, 54], F32, space="PSUM", tag="ps2")
    for c in range(4):
        for kd in range(2):
            nc.tensor.matmul(pP2[:], news[:, kd, :, c],
                             wcs[:, kd, c // 2, (c % 2) * 54:(c % 2) * 54 + 54],
                             start=(c == 0 and kd == 0), stop=(c == 3 and kd == 1))
    constt = pers.tile([16, 14], F32, tag="constt")
    for blk, fs in enumerate(FS):
        nc.vector.reduce_sum(
            constt[:, MOFF[blk]:MOFF[blk] + fs],
            pP2[:, BLKOFF[blk]:BLKOFF[blk] + fs * fs].rearrange(
                "p (a b) -> p a b", a=fs),
            axis=mybir.AxisListType.X)
    nc.vector.tensor_tensor(constt[:], constt[:], cb14s[:], OP.add)
    # transposed (o, b) for the post-maxpool add
    pCT = psum.tile([14, 16], F32, space="PSUM", tag="ps2")
    nc.tensor.transpose(pCT[:], constt[:], identf[0:16, 0:16])
    constT = pers.tile([14, 16], F32, tag="constT")
    nc.vector.tensor_copy(constT[:], pCT[:])

    # ---- P1 + Q (a-weighted token features for the conv)
    a_rep = [pers.tile([108, BS, L], BF, tag=f"nbs{ct}", name=f"arept{ct}") for ct in range(2)]
    for ct in range(2):
        for b in range(BS):
            stage = tp.tile([2, 2, 128], F32, tag="astage")
            r0 = ct * 32 + b
            nc.sync.dma_start(stage[:], aT_cb[r0:r0 + 17:16, :, :])
            pAR = psum.tile([108, L], F32, space="PSUM", tag="ps2", name="pAR")
            nc.tensor.matmul(pAR[:], binds[:],
                             stage[:].rearrange("p a b -> p (a b)"),
                             start=True, stop=True)
            nc.vector.tensor_copy(a_rep[ct][:, b, :], pAR[:])
    Q = [pers.tile([108, BS, NQ], BF, tag=f"hs0{'fb'[ct]}", name=f"Qt{ct}") for ct in range(2)]
    for ct in range(2):
        nc.gpsimd.memset(Q[ct][:, :, L:NQ], -1000.0)
        for bp in range(8):
            pQ = psum.tile([108, 2, L], F32, space="PSUM", tag="ps2")
            for kd in range(2):
                nc.tensor.matmul(pQ[:], wcs[:, kd, ct, :],
                                 x2[kd][:, 2 * bp:2 * bp + 2, :],
                                 start=(kd == 0), stop=(kd == 1))
            nc.vector.tensor_tensor(Q[ct][:, 2 * bp:2 * bp + 2, 0:L], pQ[:],
                                    a_rep[ct][:, 2 * bp:2 * bp + 2, :], OP.mult)

    # ---- conv Y via shifted selection matmuls + max pool + fc
    maxy = pers.tile([14, 16], F32, tag="maxy")
    for bp in range(8):
        pY = psum.tile([14, 2, 255], F32, space="PSUM", tag="ps2")
        for i_, (di, ct) in enumerate([(di, ct) for di in range(5) for ct in range(2)]):
            nc.tensor.matmul(pY[:], sels[0:108, di, ct, :],
                             Q[ct][:, 2 * bp:2 * bp + 2, di:di + 255],
                             start=(i_ == 0), stop=(i_ == 9))
        # windows past each block's valid range contain -1000 Q-padding terms,
        # so a full-width max is safe for every block
        nc.vector.reduce_max(maxy[:, 2 * bp:2 * bp + 2], pY[:],
                             axis=mybir.AxisListType.X)
    # y includes conv(a*x); add conv(new)+bias (constant over i) after the max,
    # then relu — both commute with the max since relu is monotone.
    nc.vector.tensor_tensor(maxy[:], maxy[:], constT[:], OP.add)
    nc.vector.tensor_scalar_max(maxy[:], maxy[:], 0.0)
    if DEBUG:
        nc.sync.dma_start(P['d_maxy'][:], maxy[:])
    feats = pers.tile([16, 16], BF, tag="feats")
    nc.gpsimd.memset(feats[:], 0.0)
    nc.vector.tensor_copy(feats[0:14, :], maxy[:])
    pfc = psum.tile([2, 16], F32, space="PSUM", tag="ps2")
    nc.tensor.matmul(pfc[:], fcws[:], feats[:], start=True, stop=True)
    outs = pers.tile([2, 16], F32, tag="outs")
    nc.vector.tensor_scalar_add(outs[:], pfc[:], fcbs[:, 0:1])
    nc.sync.dma_start(P['out'][:], outs[:])


# ---------------------------------------------------------------- host
def _prep_shared(inputs):
    f32 = np.float32
    emb16 = np.zeros((V, EP), dtype=bf16)
    emb16[:, :E] = inputs['emb'].astype(bf16)

    w0t = np.zeros((128, 2, 3, EP), dtype=bf16)
    wih0 = inputs['w_ih0'].astype(f32)          # (2, 384, 300)
    for k in range(3):
        lo, hi = k * 128, min((k + 1) * 128, E)
        w0t[0:hi - lo, :, k, :] = np.transpose(wih0[:, :, lo:hi], (2, 0, 1)).astype(bf16)

    w1t = np.zeros((128, 2, 2, EP), dtype=bf16)
    wih1 = inputs['w_ih1'].astype(f32)          # (2, 384, 256)
    for k in range(2):
        w1t[:, :, k, :] = np.transpose(wih1[:, :, k * 128:(k + 1) * 128],
                                       (2, 0, 1)).astype(bf16)

    whh = np.zeros((128, 2, 2, 3, 128), dtype=bf16)
    for l, wname in ((0, 'w_hh0'), (1, 'w_hh1')):
        w = inputs[wname].astype(f32)           # (2, 384, 128)
        for g in range(3):
            # [p, d, m] = w[d, g*128+m, p]
            whh[:, l, :, g, :] = np.transpose(w[:, g * 128:(g + 1) * 128, :],
                                              (2, 0, 1)).astype(bf16)

    gb = np.zeros((128, 2, 2, 4), dtype=f32)
    for l, (bi, bh) in ((0, ('b_ih0', 'b_hh0')), (1, ('b_ih1', 'b_hh1'))):
        bih, bhh = inputs[bi].astype(f32), inputs[bh].astype(f32)   # (2, 384)
        gb[:, l, :, 0] = (bih[:, 0:128] + bhh[:, 0:128]).T
        gb[:, l, :, 1] = (bih[:, 128:256] + bhh[:, 128:256]).T
        gb[:, l, :, 2] = bhh[:, 256:384].T
        gb[:, l, :, 3] = bih[:, 256:384].T

    awT = np.transpose(inputs['att_w'].astype(f32).reshape(4, 2, 128, 256),
                       (2, 0, 1, 3)).astype(bf16)
    av2T = np.transpose(inputs['att_wv2'].astype(f32).reshape(4, 2, 128, 256),
                        (2, 0, 1, 3)).astype(bf16)
    abv = np.transpose(inputs['att_bv'].astype(f32).reshape(4, 2, 128), (2, 0, 1)).copy()
    abc = np.broadcast_to(inputs['att_b'].astype(f32)[None, :], (128, 4)).copy()
    wv1 = np.transpose(inputs['att_wv1'].astype(f32).reshape(4, 2, 128),
                       (2, 0, 1)).astype(bf16)

    wc = np.zeros((128, 2, 2, 108), dtype=bf16)
    sel = np.zeros((128, 5, 2, 14), dtype=bf16)
    for blk, fs in enumerate(FS):
        w = inputs[f'conv_w{blk}'].astype(f32)      # (fs, C, fs, 256)
        for c in range(4):
            ct, c_loc = c // 2, c % 2
            for o in range(fs):
                for di in range(fs):
                    m = c_loc * 54 + BLKOFF[blk] + o * fs + di
                    vec = w[o, c, di, :]            # (256,)
                    for kd in range(2):
                        wc[:, kd, ct, m] = vec[kd * 128:(kd + 1) * 128].astype(bf16)
                    sel[m, di, :, MOFF[blk] + o] = 1.0

    cb14 = np.zeros((16, 14), dtype=f32)
    for blk, fs in enumerate(FS):
        cb14[:, MOFF[blk]:MOFF[blk] + fs] = inputs[f'conv_b{blk}'].astype(f32)[None, :]

    fcw = np.zeros((16, 2), dtype=bf16)
    fcw[0:14, :] = inputs['fc_w'].astype(f32).T.astype(bf16)
    fcb = inputs['fc_b'].astype(f32).reshape(2, 1)

    identb = np.eye(128, dtype=np.float32).astype(bf16)
    identf = np.eye(128, dtype=np.float32)
    ones1 = np.ones((128, 1), dtype=f32)
    onesr = np.ones((1, 128), dtype=f32)
    bind = np.zeros((2, 108), dtype=f32)
    bind[0, 0:54] = 1.0
    bind[1, 54:108] = 1.0

    return dict(emb16=emb16, w0t=w0t, w1t=w1t, whh=whh, gb=gb, awT=awT,
                av2T=av2T, abv=abv, abc=abc, wv1=wv1, wc=wc, sel=sel,
                cb14=cb14, fcw=fcw, fcb=fcb, identb=identb, identf=identf,
                ones1=ones1, onesr=onesr, bind=bind)


def kernel(**inputs):
    global _last_debug
    shared = _prep_shared(inputs)
    utt = np.asarray(inputs['utterance']).astype(np.int32)   # (128, 256)
    in_maps = []
    for core in range(NCORES):
        m = dict(shared)
        m['idx'] = utt[core * BS:(core + 1) * BS].reshape(NTOK, 1).copy()
        in_maps.append(m)
    nc = _build_nc()
    global _last_exec_ns
    res = run_bass_kernel_spmd(nc, in_maps, list(range(NCORES)), trace=TRACE)
    _last_exec_ns = res.exec_time_ns
    out = np.zeros((B, OUT), np.float32)
    for core in range(NCORES):
        out[core * BS:(core + 1) * BS] = res.results[core]['out'].T
    if DEBUG:
        _last_debug = [res.results[core] for core in range(NCORES)]
    return out

